# revision 86
# baseline (speedup 1.0000x reference)
"""Single-head full-attention layer on 8 Trainium2 NeuronCores (fp8 DoubleRow).

reference:
    q = seq @ Wq; k = seq @ Wk; v = seq @ Wv          # [B,S,D], D=1024
    scores = q @ k.T / sqrt(D)                        # [B,S,S]
    out = seq + softmax(scores) @ v * mask            # [B,S,D]

Sharding: 8 cores = 4 batches x 2 query-halves, ZERO collectives. Two
host-side weight folds remove all duplicated projection work:
  scores = seq (Wq Wk^T) seq^T   -> A = Wq @ Wk.T precomputed on host;
                                    only the own-query projection qA =
                                    seq_own @ A runs on device, and the
                                    key side uses seq^T directly.
  attn @ (seq @ Wv) = (attn @ seq) @ Wv -> U = attn @ seq first (keys
                                    contract against raw seq), then a
                                    single [d,d] matmul by Wv*mask.
Per-core matmul work: qA (1.07 GMAC) + scores (2.15) + U (2.15) +
U@Wvm (1.07) = 6.45 GMAC, the communication-free minimum for this
decomposition.

Each core's key columns are permuted (own half first) on the host so the
SPMD program is identical on all cores; softmax sums are permutation-
invariant so the output is unchanged.

All matmuls run in fp8(E4M3) with perf_mode=DoubleRow. Numerics:
  - host scales A and Wv*mask by 32 before the fp8 cast; seq is cast
    unscaled (values ~N(0,1)).
  - exp arg = psum/1024 - 3 (scale folds A's 32 and 1/sqrt(D)=32; the
    shift keeps attn weights and U inside fp8e4's +-240 range).
  - U is scaled by 1/4 at the PSUM->fp8 copy; the colsum matmul uses
    8.0-valued ones (= 32 * 1/4) so 1/colsum absorbs every scale.
  - per output tile, normalize + residual-add run either as one DVE
    scalar_tensor_tensor, or split into an ACT scale-copy plus an
    all-bf16 DVE add (per-tile engs spec, swept in the timeline sim) to
    keep both engines off the critical tail. Residual input and output
    are bf16 (upcast on host).

Schedule: one PSUM tile = 2 banks [128, 2, 512]; exps and copies move
both banks per instruction to amortize engine overheads. Phase order
qa(0) -> scores(0) -> scores(1) -> U(1) -> out(2 halves), with qa(1)
pairs interleaved into scores(0) and U(0) pairs into scores(1): the
exp-free tiles give the ACT exp stream (1038ns/pair vs 854ns of PE
work) enough slack that the 3-deep PSUM rotation never stalls the PE.
Colsum matmuls also ride the scores streams. Input DMAs are staged so
the qa phase streams as chunks land (the first matmul is input-bytes
bound at ~4.4us); ~700 warmup matmuls ramp the PE p-state during the
initial DMA wait. The out phases keep one SBUF tile per output chunk
(p_o bufs=8) — fewer buffers transitively stall out(1)'s normalize on
out(0)'s output DMAs.
"""

import numpy as np
import ml_dtypes

import concourse.bass as bass
import concourse.mybir as mybir
import concourse.tile as tile
from concourse import bacc, bass_utils

B, S, D = 4, 2048, 1024
N_CORES = 8
SH = S // 2          # queries per core
PD = 128             # partition dim
KD = D // PD         # 8 ksub chunks over d
KH = SH // PD        # 8 chunks over own queries
KC = S // PD         # 16 ksub chunks over all keys
NT = 512             # matmul free-dim tile (one PSUM bank of fp32)
NQ = SH // NT        # 2 query 512-halves per core
F8 = mybir.dt.float8e4
F32 = mybir.dt.float32
BF16 = mybir.dt.bfloat16
W_SCALE = 32.0
U_SCALE = 0.25
EXP_SCALE = 1.0 / (W_SCALE * 32.0)     # psum = 32*score_raw; /sqrt(D)=32
EXP_SHIFT = -3.0
CS_ONES = W_SCALE * U_SCALE            # 8.0: colsum absorbs U and Wv scales
DR = mybir.MatmulPerfMode.DoubleRow
NWU = 775                              # PE warmup matmuls (pstate ramp + DMA wait)

_FP8 = ml_dtypes.float8_e4m3
_BF16 = ml_dtypes.bfloat16


def _build_kernel(tc):
    nc = tc.nc
    seqTr = nc.dram_tensor("seqTr", [PD, KD, S], F8, kind="ExternalInput").ap()
    seqNr = nc.dram_tensor("seqNr", [PD, KC, D], F8, kind="ExternalInput").ap()
    # a3 is column-block-major [p, block, ksub, 128]: only qa reads it, and
    # this layout makes every 2-block DMA piece contiguous on BOTH sides
    # (728ns per piece instead of 1456ns), taking A off the critical path
    a3d = nc.dram_tensor("a3d", [PD, KD, KD, PD], F8, kind="ExternalInput").ap()
    wv3d = nc.dram_tensor("wv3d", [PD, KD, D], F8, kind="ExternalInput").ap()
    sh3d = nc.dram_tensor("sh3d", [PD, KH, D], BF16, kind="ExternalInput").ap()
    outT = nc.dram_tensor("outT", [SH, D], BF16, kind="ExternalOutput").ap()

    Exp = mybir.ActivationFunctionType.Exp
    Copy = mybir.ActivationFunctionType.Copy

    with (
        tc.tile_pool(name="p_seq", bufs=1) as p_seq,
        tc.tile_pool(name="p_seqn", bufs=1) as p_seqn,
        tc.tile_pool(name="p_a", bufs=1) as p_a,
        tc.tile_pool(name="p_wv", bufs=1) as p_wv,
        tc.tile_pool(name="p_qt", bufs=1) as p_qt,
        tc.tile_pool(name="p_at", bufs=1) as p_at,
        tc.tile_pool(name="p_u", bufs=1) as p_u,
        tc.tile_pool(name="p_sh", bufs=1) as p_sh,
        tc.tile_pool(name="p_o", bufs=8) as p_o,
        tc.tile_pool(name="p_msc", bufs=1) as p_msc,
        tc.tile_pool(name="p_dram", bufs=1, space="DRAM") as p_dram,
        tc.tile_pool(name="p_mm", bufs=3, space="PSUM") as p_mm,
        tc.tile_pool(name="p_cs", bufs=1, space="PSUM") as p_cs,
    ):
        # ---- resident inputs, ordered by first use --------------------------
        seq3 = p_seq.tile([PD, KD, S], F8, tag="seq", name="seq3")
        seqN3 = p_seqn.tile([PD, KC, D], F8, tag="seqn", name="seqN3")
        a3 = p_a.tile([PD, KD, KD, PD], F8, tag="a", name="a3")
        wv3 = p_wv.tile([PD, KD, D], F8, tag="wv", name="wv3")
        sh3 = p_sh.tile([PD, KH, D], BF16, tag="sh", name="sh3")

        # qA phase (n=0) consumes a3 column chunks in m order and seq3 query
        # chunks in n order; stage the DMAs so each lands just in time.
        nc.sync.dma_start(a3[:, 0:2], a3d[:, 0:2])
        nc.sync.dma_start(seq3[:, :, 0:NT], seqTr[:, :, 0:NT])
        nc.sync.dma_start(a3[:, 2:4], a3d[:, 2:4])
        nc.sync.dma_start(a3[:, 4:6], a3d[:, 4:6])
        nc.sync.dma_start(a3[:, 6:8], a3d[:, 6:8])
        nc.sync.dma_start(seq3[:, :, NT:SH], seqTr[:, :, NT:SH])
        nc.sync.dma_start(seq3[:, :, SH:S], seqTr[:, :, SH:S])
        nc.sync.dma_start(seqN3[:, 0:KH, :], seqNr[:, 0:KH, :])
        nc.sync.dma_start(seqN3[:, KH:KC, :], seqNr[:, KH:KC, :])
        nc.sync.dma_start(wv3[:], wv3d[:])
        nc.sync.dma_start(sh3[:], sh3d[:])

        # ---- HAM warm-up: keep the PE busy on dummy matmuls during the
        # input-DMA wait so the clock gate is at 2.4GHz when real work starts
        wu_sb = p_msc.tile([PD, 2, 16], F8, tag="wu", name="wu_sb")
        nc.vector.memset(wu_sb[:], 0.0)
        ps_wu = p_mm.tile([PD, 2, NT], F32, tag="mm", name="ps_wu")
        for i in range(NWU):
            nc.tensor.matmul(
                ps_wu[0:16, 0, 0:16], wu_sb[:, 0:2, 0:16], wu_sb[:, 0:2, 0:16],
                start=(i == 0), stop=(i == NWU - 1), perf_mode=DR,
            )

        # ---- small constants ------------------------------------------------
        ones3 = p_msc.tile([PD, 2, 16], F8, tag="ones", name="ones3")
        nc.vector.memset(ones3[:], CS_ONES)
        ebias = p_msc.tile([PD, 1], F32, tag="ebias", name="ebias")
        nc.vector.memset(ebias[:], EXP_SHIFT)

        qAt3 = p_qt.tile([PD, KD, SH], F8, tag="qt", name="qAt3")
        at3 = p_at.tile([PD, KC, SH], F8, tag="at", name="at3")
        u3T = p_u.tile([PD, KD, SH], F8, tag="u", name="u3T")
        cs_ps = p_cs.tile([1, SH], F32, tag="cs", name="cs")
        cs_sb = p_msc.tile([1, SH], F32, tag="cs_sb", name="cs_sb")
        cs_d = [
            p_dram.tile([1, NT], F32, tag=f"csd{n}", name=f"cs_d{n}")
            for n in range(NQ)
        ]
        csT = p_msc.tile([PD, KH], F32, tag="csT", name="csT")
        recipT = p_msc.tile([PD, KH], F32, tag="recipT", name="recipT")

        # ---- phase 1: qAt = (seq_own @ A).T in [d_out, q] layout ------------
        # m-chunks processed in PAIRS sharing one 2-bank PSUM tile; the
        # PSUM->fp8 copy moves both banks in one instruction (amortizes the
        # per-instruction engine overhead).
        def qa_pair(n, m, dve_copy=False):
            ps = p_mm.tile([PD, 2, NT], F32, tag="mm", name=f"ps_q{n}_{m}")
            for mm_ in range(2):
                for k in range(0, KD, 2):
                    nc.tensor.matmul(
                        ps[:, mm_, :],
                        a3[:, m + mm_, k:k + 2, :],
                        seq3[:, k:k + 2, n * NT:(n + 1) * NT],
                        start=(k == 0), stop=(k == KD - 2), perf_mode=DR,
                    )
            # alternate copy engines so neither ACT nor DVE paces the PE;
            # dve_copy forces DVE when ACT is busy with the exp stream
            if (m // 2) % 2 == 0 and not dve_copy:
                nc.scalar.activation(qAt3[:, m:m + 2, n * NT:(n + 1) * NT],
                                     ps[:], Copy)
            else:
                nc.vector.tensor_copy(qAt3[:, m:m + 2, n * NT:(n + 1) * NT],
                                      ps[:])

        def qa_half(n):
            for m in range(0, KD, 2):
                qa_pair(n, m)

        # ---- phase 2: at = exp(scores/1024 - 3); colsum via ones matmul -----
        def colsum_mm(n, mp):
            nc.tensor.matmul(
                cs_ps[:, n * NT:(n + 1) * NT],
                ones3[:, 0:2, 0:1],
                at3[:, mp:mp + 2, n * NT:(n + 1) * NT],
                start=(mp == 0), stop=(mp == KC - 2), perf_mode=DR,
            )

        def scores_pair(n, m):
            ps = p_mm.tile([PD, 2, NT], F32, tag="mm", name=f"ps_s{n}_{m}")
            for mm_ in range(2):
                for k in range(0, KD, 2):
                    nc.tensor.matmul(
                        ps[:, mm_, :],
                        seq3[:, k:k + 2, (m + mm_) * PD:(m + mm_ + 1) * PD],
                        qAt3[:, k:k + 2, n * NT:(n + 1) * NT],
                        start=(k == 0), stop=(k == KD - 2), perf_mode=DR,
                    )
            nc.scalar.activation(
                at3[:, m:m + 2, n * NT:(n + 1) * NT], ps[:], Exp,
                bias=ebias[:], scale=EXP_SCALE,
            )
            # colsum pair (m-4, m-3) emitted late so PE never waits on exp
            if m >= 4:
                colsum_mm(n, m - 4)

        def scores_half(n, m_lo=0, m_hi=KC):
            for m in range(m_lo, m_hi, 2):
                scores_pair(n, m)

        def colsum_finish(n):
            colsum_mm(n, KC - 4)
            colsum_mm(n, KC - 2)
            nc.vector.tensor_copy(cs_sb[:, n * NT:(n + 1) * NT],
                                  cs_ps[:, n * NT:(n + 1) * NT])
            nc.gpsimd.dma_start(cs_d[n][:], cs_sb[:, n * NT:(n + 1) * NT])
            # transpose colsum to per-partition [128, 4] via a DRAM bounce
            nc.gpsimd.dma_start(
                csT[:, n * 4:(n + 1) * 4],
                cs_d[n].rearrange("o (m p) -> (o p) m", p=PD),
            )
            nc.vector.reciprocal(recipT[:, n * 4:(n + 1) * 4],
                                 csT[:, n * 4:(n + 1) * 4])

        # ---- phase 3: uT = (attn_unnorm @ seq).T / 4 in [d, q] layout -------
        # per-m copies right after each m's matmuls, alternating ACT/DVE, so
        # the next phase's first tile never waits on a wide trailing copy
        def u_pair(n, m, dve_only=False):
            # dve_only: when interleaved with the exp stream, ACT is busy, so
            # both copies go to DVE to keep the psum rotation moving
            ps = p_mm.tile([PD, 2, NT], F32, tag="mm", name=f"ps_u{n}_{m}")
            for mm_ in range(2):
                for k in range(0, KC, 2):
                    nc.tensor.matmul(
                        ps[:, mm_, :],
                        seqN3[:, k:k + 2, (m + mm_) * PD:(m + mm_ + 1) * PD],
                        at3[:, k:k + 2, n * NT:(n + 1) * NT],
                        start=(k == 0), stop=(k == KC - 2), perf_mode=DR,
                    )
                dst = u3T[:, m + mm_, n * NT:(n + 1) * NT]
                if mm_ == 0 and not dve_only:
                    nc.scalar.activation(dst, ps[:, 0, :], Copy, scale=U_SCALE)
                else:
                    nc.vector.tensor_scalar_mul(dst, ps[:, mm_, :], U_SCALE)

        def u_half(n):
            for m in range(0, KD, 2):
                u_pair(n, m)

        # ---- phase 4: out = (uT.T @ Wvm) * recip[q] + seq, per q-chunk ------
        # Normalize+residual per tile either as one DVE stt, or split into an
        # ACT scale-copy plus a cheap all-bf16 add on DVE or the idle Pool
        # engine; engs picks per-tile paths so DVE/ACT are clear when the
        # final tiles' psums land.
        def out_tile(n, j, eng):
            if True:
                m = n * 4 + j
                o_t = p_o.tile([PD, D], BF16, tag="o", name=f"o{m}")
                ps = p_mm.tile([PD, 2, NT], F32, tag="mm", name=f"ps_o{m}")
                for nn in range(D // NT):
                    for k in range(0, KD, 2):
                        nc.tensor.matmul(
                            ps[:, nn, :],
                            u3T[:, k:k + 2, m * PD:(m + 1) * PD],
                            wv3[:, k:k + 2, nn * NT:(nn + 1) * NT],
                            start=(k == 0), stop=(k == KD - 2), perf_mode=DR,
                        )
                if eng == "stt":
                    nc.vector.scalar_tensor_tensor(
                        o_t[:],
                        ps[:],
                        recipT[:, m:m + 1],
                        sh3[:, m, :],
                        op0=mybir.AluOpType.mult,
                        op1=mybir.AluOpType.add,
                    )
                else:
                    nc.scalar.activation(o_t[:], ps[:], Copy,
                                         scale=recipT[:, m:m + 1])
                    add_eng = nc.gpsimd if eng == "act_pool" else nc.vector
                    add_eng.tensor_add(o_t[:], o_t[:], sh3[:, m, :])
                nc.sync.dma_start(outT[m * PD:(m + 1) * PD, :], o_t[:])

        qa_half(0)
        # interleave qa(1) pairs into the scores(0) stream — same trick as
        # the scores(1)/u(0) interleave below: exp-free tiles give the ACT
        # exp stream slack so the PE never stalls on the psum rotation
        qa_pair(1, 0)
        qa_pair(1, 2, dve_copy=True)
        scores_pair(0, 0)
        scores_pair(0, 2)
        qa_pair(1, 4, dve_copy=True)
        scores_pair(0, 4)
        scores_pair(0, 6)
        qa_pair(1, 6, dve_copy=True)
        scores_pair(0, 8)
        scores_pair(0, 10)
        scores_pair(0, 12)
        scores_pair(0, 14)
        # interleave u(0) pairs into the scores(1) stream: u tiles have no
        # exp dependency, so they give the ACT exp stream slack and the PE
        # never stalls on the psum rotation
        scores_pair(1, 0)
        scores_pair(1, 2)
        colsum_finish(0)
        u_pair(0, 0, dve_only=True)
        scores_pair(1, 4)
        scores_pair(1, 6)
        u_pair(0, 2, dve_only=True)
        scores_pair(1, 8)
        scores_pair(1, 10)
        u_pair(0, 4, dve_only=True)
        scores_pair(1, 12)
        scores_pair(1, 14)
        u_pair(0, 6, dve_only=True)
        colsum_finish(1)
        u_half(1)
        for j, eng in enumerate(("stt", "act_dve", "stt", "act_dve")):
            out_tile(0, j, eng)
        for j, eng in enumerate(("stt", "act_dve", "stt", "act_dve")):
            out_tile(1, j, eng)


_NC_CACHE = None


def _get_nc():
    global _NC_CACHE
    if _NC_CACHE is None:
        nc = bacc.Bacc(
            "TRN2", target_bir_lowering=False, debug=False, num_devices=N_CORES
        )
        with tile.TileContext(nc) as tc:
            _build_kernel(tc)
        nc.compile()
        _NC_CACHE = nc
    return _NC_CACHE


def _chunk(x, nchunk, dtype):
    # [nchunk*PD, F] -> [PD, nchunk, F]
    n, f = x.shape
    return np.ascontiguousarray(
        x.reshape(nchunk, PD, f).transpose(1, 0, 2)).astype(dtype)


def _prep_in_maps(seq, Wq, Wk, Wv, mask):
    seq = np.asarray(seq, dtype=np.float32)
    A = (np.asarray(Wq, np.float32) @ np.asarray(Wk, np.float32).T) * W_SCALE
    Wvm = (np.asarray(Wv, np.float32)
           * np.asarray(mask, np.float32)[None, :]) * W_SCALE
    a3c = _chunk(A, KD, np.float32)            # [PD, j, D]
    a3d = np.ascontiguousarray(
        a3c.reshape(PD, KD, KD, PD).transpose(0, 2, 1, 3)).astype(_FP8)
    wv3d = _chunk(Wvm, KD, _FP8)
    in_maps = []
    for c in range(N_CORES):
        b, h = divmod(c, 2)
        own = seq[b, h * SH:(h + 1) * SH, :]
        other = seq[b, (1 - h) * SH:(2 - h) * SH, :]
        seq_perm = np.concatenate([own, other], axis=0)     # own keys first
        in_maps.append({
            "seqTr": _chunk(np.ascontiguousarray(seq_perm.T), KD, _FP8),
            "seqNr": _chunk(seq_perm, KC, _FP8),
            "a3d": a3d,
            "wv3d": wv3d,
            "sh3d": _chunk(own, KH, _BF16),
        })
    return in_maps


def _run(seq, Wq, Wk, Wv, mask, trace=False, **run_kwargs):
    nc = _get_nc()
    in_maps = _prep_in_maps(seq, Wq, Wk, Wv, mask)
    res = bass_utils.run_bass_kernel_spmd(
        nc, in_maps, core_ids=list(range(N_CORES)), trace=trace, **run_kwargs
    )
    out = np.empty((B, S, D), dtype=np.float32)
    for c in range(N_CORES):
        b, h = divmod(c, 2)
        out[b, h * SH:(h + 1) * SH, :] = res.results[c]["outT"]
    return out, res


def kernel(seq, Wq, Wk, Wv, mask):
    out, _ = _run(seq, Wq, Wk, Wv, mask)
    return out
